# revision 2
# baseline (speedup 1.0000x reference)
"""Trainium2 Bass kernel for CascadedPathEncoder.

Reference computation (per sample b):
    h_0 = relu(W_0 @ [0_256; wp_0] + b_0)
    h_p = relu(W_p @ [h_{p-1}; wp_p] + b_p)      p = 1..31
    out[b] = concat_p h_p                         -> [8192, 8192]

Strategy: pure data parallel over 8 NeuronCores (1024 batch rows each),
bf16 compute with f32 PSUM accumulation. Per core the hidden state
lives transposed in SBUF as one bf16 [128, 2, 2, 512] tile per step
(partition = hidden-within-chunk, dims = m-chunk, batch-tile, batch).

Per step the PSUM accumulation for each of the 4 output banks
(m-chunk x batch-tile) opens with the wp contribution (true K=4) and
adds two K=128 chunks of the previous hidden state. The 4 wp matmuls
run CONCURRENTLY as 32x128 row-tiles of the PE array (tile_position
(32q, 0), q = 2m+t): each tile reads its lhsT/rhs from SBUF partitions
32q..32q+3 and writes its own PSUM bank, so the whole wp pass costs
~one matmul span instead of four. This removes the 1/3 of PE conveyor
cycles the baseline burned streaming N=512 columns through a K=64
zero-padded pass.

The K=128 h matmuls run k-outer / m-inner / t-innermost (consecutive
matmuls share a weight load). Bias+relu run on the Activation engine
(m=0) and Vector engine (m=1), split per batch-tile so the next step's
k=0 matmuls wait only on a half-relu. Outputs stream per step as two
256KB DMAs: the ACT half on the scalar HWDGE ring (issued by the same
engine that produced it), the DVE half on the GpSimd SWDGE ring; the
last two steps split 3 ways (scalar/gpsimd/sync) to shorten the drain
tail. Inputs use the Sync HWDGE ring, ordered by first use.

Host re-assembles the full [8192, 8192] f32 from bf16 step outputs.
"""

import numpy as np
import ml_dtypes

BF16 = ml_dtypes.bfloat16

P = 32          # scan steps
PD = 4          # point dim
H = 256         # hidden dim
B = 8192        # global batch
NCORES = 8
BS = B // NCORES  # 1024 rows per core
TN = 512        # matmul moving free dim (one PSUM bank of f32; ISA max)
NT = BS // TN   # batch tiles per core

_CACHE = {}


def _build_nc():
    from contextlib import ExitStack

    import concourse.bass as bass
    import concourse.tile as tile
    from concourse import bacc, mybir

    dt = mybir.dt
    ts = bass.ts

    nc = bacc.Bacc(
        "TRN2", target_bir_lowering=False, debug=False, num_devices=NCORES
    )
    WH_CHUNK = 4  # steps per wh DMA chunk (pipeline weight loads)
    # wh[kk, p, k, jj] = W[p, jj + 128m, 128k + kk] (lhsT for the h chunks)
    wh = nc.dram_tensor("wh", [128, P, 2, 256], dt.bfloat16, kind="ExternalInput").ap()
    # wxr[4q + r, p, j] = W[p, 128*(q//2) + j, 256 + r]: true-K=4 lhsT
    # blocks for the row-tiled wp pass, strip q handles (m, t) = (q//2, q%2)
    wxr = nc.dram_tensor("wxr", [16, P, 128], dt.bfloat16, kind="ExternalInput").ap()
    # pdq[4q + r, p, n] = path_data[c*BS + (q%2)*TN + n, 4p + r]
    pdq = nc.dram_tensor("pdq", [16, P, TN], dt.bfloat16, kind="ExternalInput").ap()
    bias = nc.dram_tensor("bias", [128, P, 2], dt.float32, kind="ExternalInput").ap()
    out = nc.dram_tensor(
        "out", [P, 128, 2, NT, TN], dt.bfloat16, kind="ExternalOutput"
    ).ap()

    with tile.TileContext(nc) as tc, ExitStack() as ctx:
        const = ctx.enter_context(tc.tile_pool(name="const", bufs=1))
        state = ctx.enter_context(tc.tile_pool(name="state", bufs=10))
        psum = ctx.enter_context(tc.tile_pool(name="psum", bufs=2, space="PSUM"))

        wxr_sb = const.tile([128, P, 128], dt.bfloat16)
        pdq_sb = const.tile([128, P, TN], dt.bfloat16)
        b_sb = const.tile([128, P, 2], dt.float32)
        wh_sb = const.tile([128, P, 2, 256], dt.bfloat16)

        # Input DMAs on the sync HWDGE ring, ordered by first use. Only
        # partitions 32q..32q+3 of wxr_sb/pdq_sb are ever read.
        for q in range(4):
            nc.sync.dma_start(
                out=wxr_sb[32 * q : 32 * q + 4, :, :], in_=wxr[4 * q : 4 * q + 4, :, :]
            )
        for q in range(4):
            nc.sync.dma_start(
                out=pdq_sb[32 * q : 32 * q + 4, :, :], in_=pdq[4 * q : 4 * q + 4, :, :]
            )
        nc.sync.dma_start(out=wh_sb[:, 0:2, :, :], in_=wh[:, 0:2, :, :])
        nc.sync.dma_start(out=b_sb[:], in_=bias[:])
        for g in range(P // WH_CHUNK):
            sl = slice(2 + g * WH_CHUNK, min(2 + (g + 1) * WH_CHUNK, P))
            nc.sync.dma_start(out=wh_sb[:, sl, :, :], in_=wh[:, sl, :, :])

        h_prev = None
        for p in range(P):
            ps = [
                psum.tile(
                    [128, NT, TN],
                    dt.float32,
                    tag=f"ps_m{m}",
                    name=f"ps_p{p}m{m}",
                )
                for m in range(2)
            ]
            # wp pass: 4 concurrent 32x128 row-tiles, one per (m, t) bank.
            # True K=4 at base partition 32q; opens each accumulation group.
            for q in range(4):
                m, t = q // 2, q % 2
                nc.tensor.matmul(
                    ps[m][:, t, :],
                    lhsT=wxr_sb[32 * q : 32 * q + 4, p, :],
                    rhs=pdq_sb[32 * q : 32 * q + 4, p, :],
                    start=True,
                    stop=(p == 0),
                    tile_position=(32 * q, 0),
                )
            # h pass: k outer / m inner / t innermost (t pairs share lhsT).
            # k=0 needs only the ACT half of the previous relu, k=1 the DVE
            # half, giving each engine a long completion window.
            if p > 0:
                for k in range(2):
                    for m in range(2):
                        for t in range(NT):
                            nc.tensor.matmul(
                                ps[m][:, t, :],
                                lhsT=wh_sb[:, p, k, ts(m, 128)],
                                rhs=h_prev[:, k, t, :],
                                start=False,
                                stop=(k == 1),
                            )
            hn = state.tile(
                [128, 2, NT, TN], dt.bfloat16, tag="h", name=f"h_p{p}"
            )
            # bias+relu, split per batch-tile so the next step's first
            # matmuls wait only on a half-relu
            for t in range(NT):
                nc.scalar.activation(
                    hn[:, 0, t, :],
                    ps[0][:, t, :],
                    mybir.ActivationFunctionType.Relu,
                    bias=b_sb[:, p, 0:1],
                    scale=1.0,
                )
            for t in range(NT):
                nc.vector.tensor_scalar(
                    hn[:, 1, t, :],
                    ps[1][:, t, :],
                    scalar1=b_sb[:, p, 1:2],
                    scalar2=0.0,
                    op0=mybir.AluOpType.add,
                    op1=mybir.AluOpType.max,
                )
            if p >= P - 2:
                # tail: split 3 ways so the last relu halves ship in parallel
                nc.scalar.dma_start(out=out[p, :, 0, :, :], in_=hn[:, 0, :, :])
                nc.gpsimd.dma_start(out=out[p, :, 1, 0, :], in_=hn[:, 1, 0, :])
                nc.sync.dma_start(out=out[p, :, 1, 1, :], in_=hn[:, 1, 1, :])
            else:
                # ACT half on the scalar HWDGE ring (same engine produced
                # it: no cross-engine wait), DVE half on the SWDGE ring
                nc.scalar.dma_start(out=out[p, :, 0, :, :], in_=hn[:, 0, :, :])
                nc.gpsimd.dma_start(out=out[p, :, 1, :, :], in_=hn[:, 1, :, :])
            h_prev = hn

    nc.compile()
    return nc


def _get_nc():
    if "nc" not in _CACHE:
        _CACHE["nc"] = _build_nc()
    return _CACHE["nc"]


def _pack_inputs(path_data, W, b):
    """Host-side packing into the DRAM layouts the kernel expects."""
    # lhsT for the two K=128 chunks: wh[kk, p, k, jj] = W[p, jj, 128k+kk]
    wh_np = np.ascontiguousarray(
        W[:, :, :H].reshape(P, H, 2, 128).transpose(3, 0, 2, 1)
    ).astype(BF16)
    # true-K=4 lhsT blocks for the row-tiled wp pass:
    # wxr[4q+r, p, j] = W[p, 128*(q//2)+j, 256+r]
    wxs = W[:, :, H:]  # [P, 256, PD]
    wxr_np = np.empty((16, P, 128), dtype=BF16)
    for q in range(4):
        m = q // 2
        wxr_np[4 * q : 4 * q + 4] = (
            wxs[:, 128 * m : 128 * (m + 1), :].transpose(2, 0, 1).astype(BF16)
        )
    # bias[j, p, m] = b[p, 128m+j]
    b_np = np.ascontiguousarray(b.reshape(P, 2, 128).transpose(2, 0, 1)).astype(
        np.float32
    )
    # per-core rhs strips: pdq[4q+r, p, n] = path_data[c*BS + (q%2)*TN + n, 4p+r]
    pdq_all = []
    for c in range(NCORES):
        pd_c = path_data[c * BS : (c + 1) * BS].reshape(NT, TN, P, PD)
        pdq_np = np.empty((16, P, TN), dtype=BF16)
        for q in range(4):
            t = q % 2
            pdq_np[4 * q : 4 * q + 4] = pd_c[t].transpose(2, 1, 0).astype(BF16)
        pdq_all.append(pdq_np)
    return wh_np, wxr_np, b_np, pdq_all


def _make_in_maps(path_data, W, b):
    wh_np, wxr_np, b_np, pdq_all = _pack_inputs(path_data, W, b)
    return [
        {"wh": wh_np, "wxr": wxr_np, "bias": b_np, "pdq": pdq_all[c]}
        for c in range(NCORES)
    ]


def _unpack_out(results):
    # out[p, jj, m, t, bb] -> full[c*BS + t*TN + bb, p*256 + m*128 + jj]
    return np.concatenate(
        [
            np.asarray(r["out"])
            .transpose(3, 4, 0, 2, 1)
            .reshape(BS, P * H)
            .astype(np.float32)
            for r in results
        ],
        axis=0,
    )


def kernel(path_data, W, b):
    from concourse.bass_utils import run_bass_kernel_spmd

    path_data = np.asarray(path_data, dtype=np.float32)
    W = np.asarray(W, dtype=np.float32)
    b = np.asarray(b, dtype=np.float32)

    in_maps = _make_in_maps(path_data, W, b)
    nc = _get_nc()
    res = run_bass_kernel_spmd(nc, in_maps, core_ids=list(range(NCORES)))
    return _unpack_out(res.results)


# revision 3
# speedup vs baseline: 1.0633x; 1.0633x over previous
"""Trainium2 Bass kernel for CascadedPathEncoder.

Reference computation (per sample b):
    h_0 = relu(W_0 @ [0_256; wp_0] + b_0)
    h_p = relu(W_p @ [h_{p-1}; wp_p] + b_p)      p = 1..31
    out[b] = concat_p h_p                         -> [8192, 8192]

Strategy: pure data parallel over 8 NeuronCores (1024 batch rows each),
bf16 compute with f32 PSUM accumulation. Per core the hidden state
lives transposed in SBUF as one bf16 [128, 2, 2, 512] tile per step
(partition = hidden-within-chunk, dims = m-chunk, batch-tile, batch).

Per step the PSUM accumulation for each of the 4 output banks
(m-chunk x batch-tile) opens with the wp contribution and adds two
K=128 chunks of the previous hidden state. The 4 wp matmuls run
CONCURRENTLY as 32x128 row-tiles of the PE array (tile_position
(32q, 0), strip q = 2m+t): the whole wp pass costs ~one matmul span
instead of four. Each strip contracts K=32 with the zeros carried in
the WEIGHTS (wxz, 8 steps share a strip phase) so the rhs (pdq) is
fully dense and both wxz and pdq DMA at full 128-partition width --
a 4-partition strip layout would land on only 2 of the 16 SDMA
engines and crawl at ~25 GB/s (measured).

The K=128 h matmuls run t-outer, (k0,m0),(k1,m0),(k0,m1),(k1,m1)
within each batch-tile, so the four PSUM banks close at matmul slots
2/4/6/8. Bias+relu are interleaved: ACT takes the m0 banks (closing
at slots 2 and 6), DVE the m1 banks (slots 4 and 8); each k0 matmul
of the next step then waits only on an ACT product and each k1 only
on a DVE product, and the relu ring (~1.8us) hides under the PE
conveyor (~2.2us/step). Outputs stream per step as two 256KB DMAs
issued by engines with idle queues: Sync HWDGE ships the t=0 halves,
GpSimd SWDGE the t=1 halves (never the ACT engine: a HWDGE
DMA_DIRECT2D dispatch occupies it for ~0.6us, measured). Inputs use
the Sync HWDGE ring first, ordered by first use.

Host re-assembles the full [8192, 8192] f32 from bf16 step outputs.
"""

import numpy as np
import ml_dtypes

BF16 = ml_dtypes.bfloat16

P = 32          # scan steps
PD = 4          # point dim
H = 256         # hidden dim
B = 8192        # global batch
NCORES = 8
BS = B // NCORES  # 1024 rows per core
TN = 512        # matmul moving free dim (one PSUM bank of f32; ISA max)
NT = BS // TN   # batch tiles per core
NU = P // 8     # pdq phase groups (8 steps per strip phase)

_CACHE = {}


def _build_nc():
    from contextlib import ExitStack

    import concourse.bass as bass
    import concourse.tile as tile
    from concourse import bacc, mybir

    dt = mybir.dt
    ts = bass.ts

    nc = bacc.Bacc(
        "TRN2", target_bir_lowering=False, debug=False, num_devices=NCORES
    )
    # wh[kk, p, k, jj] = W[p, jj + 128m, 128k + kk] (lhsT for the h chunks)
    wh = nc.dram_tensor("wh", [128, P, 2, 256], dt.bfloat16, kind="ExternalInput").ap()
    # Row-tiled wp-pass weights, K=32 per strip with zero rows baked in:
    # wxz[32q + 4s + r, p, j] = W[p, 128*(q//2)+j, 256+r] if s == p%8 else 0
    wxz = nc.dram_tensor("wxz", [128, P, 128], dt.bfloat16, kind="ExternalInput").ap()
    # dense rhs strips: pdq[32q + 4s + r, u, n] =
    #   path_data[c*BS + (q%2)*TN + n, 4*(s + 8u) + r]
    pdq = nc.dram_tensor("pdq", [128, NU, TN], dt.bfloat16, kind="ExternalInput").ap()
    bias = nc.dram_tensor("bias", [128, P, 2], dt.float32, kind="ExternalInput").ap()
    out = nc.dram_tensor(
        "out", [P, 128, 2, NT, TN], dt.bfloat16, kind="ExternalOutput"
    ).ap()

    with tile.TileContext(nc) as tc, ExitStack() as ctx:
        const = ctx.enter_context(tc.tile_pool(name="const", bufs=1))
        state = ctx.enter_context(tc.tile_pool(name="state", bufs=10))
        psum = ctx.enter_context(tc.tile_pool(name="psum", bufs=2, space="PSUM"))

        wxz_sb = const.tile([128, P, 128], dt.bfloat16)
        pdq_sb = const.tile([128, NU, TN], dt.bfloat16)
        b_sb = const.tile([128, P, 2], dt.float32)
        wh_sb = const.tile([128, P, 2, 256], dt.bfloat16)

        # Input DMAs on the sync HWDGE ring, ordered by first use; all
        # transfers are full 128-partition width.
        nc.sync.dma_start(out=wxz_sb[:, 0:2, :], in_=wxz[:, 0:2, :])
        nc.sync.dma_start(out=pdq_sb[:, 0, :], in_=pdq[:, 0, :])
        nc.sync.dma_start(out=wh_sb[:, 0:2, :, :], in_=wh[:, 0:2, :, :])
        nc.sync.dma_start(out=b_sb[:], in_=bias[:])
        nc.sync.dma_start(out=wxz_sb[:, 2:8, :], in_=wxz[:, 2:8, :])
        nc.sync.dma_start(out=wh_sb[:, 2:6, :, :], in_=wh[:, 2:6, :, :])
        nc.sync.dma_start(out=pdq_sb[:, 1:NU, :], in_=pdq[:, 1:NU, :])
        nc.sync.dma_start(out=wxz_sb[:, 8:20, :], in_=wxz[:, 8:20, :])
        nc.sync.dma_start(out=wh_sb[:, 6:10, :, :], in_=wh[:, 6:10, :, :])
        nc.sync.dma_start(out=wxz_sb[:, 20:32, :], in_=wxz[:, 20:32, :])
        for g in range(5):
            sl = slice(10 + g * 5, min(10 + (g + 1) * 5, P))
            nc.sync.dma_start(out=wh_sb[:, sl, :, :], in_=wh[:, sl, :, :])

        h_prev = None
        for p in range(P):
            ps = [
                psum.tile(
                    [128, NT, TN],
                    dt.float32,
                    tag=f"ps_m{m}",
                    name=f"ps_p{p}m{m}",
                )
                for m in range(2)
            ]
            # wp pass: 4 concurrent 32x128 row-tiles, one per (m, t) bank;
            # K=32 per strip, zero weight rows select step p within the
            # strip's 8-step phase. Opens each accumulation group.
            for q in range(4):
                m, t = q // 2, q % 2
                nc.tensor.matmul(
                    ps[m][:, t, :],
                    lhsT=wxz_sb[32 * q : 32 * q + 32, p, :],
                    rhs=pdq_sb[32 * q : 32 * q + 32, p // 8, :],
                    start=True,
                    stop=(p == 0),
                    tile_position=(32 * q, 0),
                )
            hn = state.tile(
                [128, 2, NT, TN], dt.bfloat16, tag="h", name=f"h_p{p}"
            )

            def relu_act(t):
                nc.scalar.activation(
                    hn[:, 0, t, :],
                    ps[0][:, t, :],
                    mybir.ActivationFunctionType.Relu,
                    bias=b_sb[:, p, 0:1],
                    scale=1.0,
                )

            def relu_dve(t):
                nc.vector.tensor_scalar(
                    hn[:, 1, t, :],
                    ps[1][:, t, :],
                    scalar1=b_sb[:, p, 1:2],
                    scalar2=0.0,
                    op0=mybir.AluOpType.add,
                    op1=mybir.AluOpType.max,
                )

            if p > 0:
                # t-outer; per t: (k0,m0),(k1,m0),(k0,m1),(k1,m1) so banks
                # close at slots 2/4/6/8; relu fires as each bank closes.
                # k0 matmuls consume ACT products, k1 consume DVE products.
                for t in range(NT):
                    for m in range(2):
                        for k in range(2):
                            nc.tensor.matmul(
                                ps[m][:, t, :],
                                lhsT=wh_sb[:, p, k, ts(m, 128)],
                                rhs=h_prev[:, k, t, :],
                                start=False,
                                stop=(k == 1),
                            )
                        if m == 0:
                            relu_act(t)
                        else:
                            relu_dve(t)
            else:
                for t in range(NT):
                    relu_act(t)
                    relu_dve(t)
            # outputs: t=0 halves on the idle Sync ring, t=1 on SWDGE;
            # the last two steps split 3 ways (+scalar) to cut the drain
            if p >= P - 2:
                nc.sync.dma_start(out=out[p, :, :, 0, :], in_=hn[:, :, 0, :])
                nc.gpsimd.dma_start(out=out[p, :, 0, 1, :], in_=hn[:, 0, 1, :])
                nc.scalar.dma_start(out=out[p, :, 1, 1, :], in_=hn[:, 1, 1, :])
            else:
                nc.sync.dma_start(out=out[p, :, :, 0, :], in_=hn[:, :, 0, :])
                nc.gpsimd.dma_start(out=out[p, :, :, 1, :], in_=hn[:, :, 1, :])
            h_prev = hn

    nc.compile()
    return nc


def _get_nc():
    if "nc" not in _CACHE:
        _CACHE["nc"] = _build_nc()
    return _CACHE["nc"]


def _pack_inputs(path_data, W, b):
    """Host-side packing into the DRAM layouts the kernel expects."""
    # lhsT for the two K=128 chunks: wh[kk, p, k, jj] = W[p, jj, 128k+kk]
    wh_np = np.ascontiguousarray(
        W[:, :, :H].reshape(P, H, 2, 128).transpose(3, 0, 2, 1)
    ).astype(BF16)
    # K=32 strip weights with zero rows selecting the step phase:
    # wxz[32q + 4s + r, p, j] = W[p, 128*(q//2)+j, 256+r] if s == p%8 else 0
    wxz_np = np.zeros((128, P, 128), dtype=BF16)
    for q in range(4):
        m = q // 2
        for p in range(P):
            s = p % 8
            wxz_np[32 * q + 4 * s : 32 * q + 4 * s + 4, p, :] = (
                W[p, 128 * m : 128 * (m + 1), H:].T.astype(BF16)
            )
    # bias[j, p, m] = b[p, 128m+j]
    b_np = np.ascontiguousarray(b.reshape(P, 2, 128).transpose(2, 0, 1)).astype(
        np.float32
    )
    # dense per-core rhs strips:
    # pdq[32q + 4s + r, u, n] = path_data[c*BS + (q%2)*TN + n, 4*(s+8u) + r]
    pdq_all = []
    for c in range(NCORES):
        pd_c = path_data[c * BS : (c + 1) * BS].reshape(NT, TN, P, PD)
        pdq_np = np.empty((128, NU, TN), dtype=BF16)
        for q in range(4):
            t = q % 2
            blk = pd_c[t].transpose(1, 2, 0).reshape(NU, 8, PD, TN)
            pdq_np[32 * q : 32 * q + 32] = (
                blk.transpose(1, 2, 0, 3).reshape(32, NU, TN).astype(BF16)
            )
        pdq_all.append(pdq_np)
    return wh_np, wxz_np, b_np, pdq_all


def _make_in_maps(path_data, W, b):
    wh_np, wxz_np, b_np, pdq_all = _pack_inputs(path_data, W, b)
    return [
        {"wh": wh_np, "wxz": wxz_np, "bias": b_np, "pdq": pdq_all[c]}
        for c in range(NCORES)
    ]


def _unpack_out(results):
    # out[p, jj, m, t, bb] -> full[c*BS + t*TN + bb, p*256 + m*128 + jj]
    return np.concatenate(
        [
            np.asarray(r["out"])
            .transpose(3, 4, 0, 2, 1)
            .reshape(BS, P * H)
            .astype(np.float32)
            for r in results
        ],
        axis=0,
    )


def kernel(path_data, W, b):
    from concourse.bass_utils import run_bass_kernel_spmd

    path_data = np.asarray(path_data, dtype=np.float32)
    W = np.asarray(W, dtype=np.float32)
    b = np.asarray(b, dtype=np.float32)

    in_maps = _make_in_maps(path_data, W, b)
    nc = _get_nc()
    res = run_bass_kernel_spmd(nc, in_maps, core_ids=list(range(NCORES)))
    return _unpack_out(res.results)


# revision 4
# speedup vs baseline: 1.1992x; 1.1278x over previous
"""Trainium2 Bass kernel for CascadedPathEncoder.

Reference computation (per sample b):
    h_0 = relu(W_0 @ [0_256; wp_0] + b_0)
    h_p = relu(W_p @ [h_{p-1}; wp_p] + b_p)      p = 1..31
    out[b] = concat_p h_p                         -> [8192, 8192]

Strategy: pure data parallel over 8 NeuronCores (1024 batch rows each),
bf16 compute with f32 PSUM accumulation. Per core the hidden state
lives transposed in SBUF as one bf16 [128, 2, 2, 512] tile per step
(partition = hidden-within-chunk, dims = m-chunk, batch-tile, batch).

Per step each of the 4 PSUM banks (m-chunk x batch-tile) accumulates
12 full 128x128-mode matmuls: a K=128 wp matmul whose lhsT carries
zero rows for every step but p (wx packs all 32 steps' 4 wp rows into
the 128 partitions; zeros select the step), then two K=128 chunks of
the previous hidden state. Everything stays in one PE array mode:
measured, 32x128 row-tiling the wp pass is faster on paper (one
concurrent 4-tile span instead of 4 serial matmuls) but the Tile
scheduler hoists the next step's wp tiles into the middle of the
h-chain, paying 4 mode-switch drains per step -- a net loss. The
all-full-mode form is hoist-immune: the conveyor just streams.

The h matmuls run t-outer, (k0,m0),(k1,m0),(k0,m1),(k1,m1) within
each batch-tile, so banks close at slots 2/4/6/8 of the h-chain.
Bias+relu interleave: ACT takes the m0 banks (slots 2, 6), DVE the m1
banks (slots 4, 8); next-step k0 matmuls then wait only on an ACT
product and k1 only on a DVE product, hiding the relu ring under the
PE conveyor (~2.6us/step). ~120 warmup matmuls on a memset tile run
while inputs load, so the HAM clock gate (PE at 1.2 GHz until ~3.4us
of sustained activity) flips before the first real matmul. Outputs
stream per step as two 256KB DMAs from idle queues: Sync HWDGE ships
the t=0 halves, GpSimd SWDGE the t=1 halves (never the ACT engine: a
HWDGE dispatch occupies it ~0.6us). The last two steps split 3 ways
(+scalar ring) to shorten the drain tail.

Host re-assembles the full [8192, 8192] f32 from bf16 step outputs.
"""

import numpy as np
import ml_dtypes

BF16 = ml_dtypes.bfloat16

P = 32          # scan steps
PD = 4          # point dim
H = 256         # hidden dim
B = 8192        # global batch
NCORES = 8
BS = B // NCORES  # 1024 rows per core
TN = 512        # matmul moving free dim (one PSUM bank of f32; ISA max)
NT = BS // TN   # batch tiles per core
NWARM = 120     # PE warmup matmuls (HAM un-throttle before first real MM)

_CACHE = {}


def _build_nc():
    from contextlib import ExitStack

    import concourse.bass as bass
    import concourse.tile as tile
    from concourse import bacc, mybir

    dt = mybir.dt
    ts = bass.ts

    nc = bacc.Bacc(
        "TRN2", target_bir_lowering=False, debug=False, num_devices=NCORES
    )
    # wh[kk, p, k, jj] = W[p, jj + 128m, 128k + kk] (lhsT for the h chunks)
    wh = nc.dram_tensor("wh", [128, P, 2, 256], dt.bfloat16, kind="ExternalInput").ap()
    # K=128 wp lhsT with zero rows selecting the step:
    # wx[4q + r, p, m, j] = W[p, 128m + j, 256 + r] if q == p else 0
    wx = nc.dram_tensor("wx", [128, P, 2, 128], dt.bfloat16, kind="ExternalInput").ap()
    # pdx[4q + r, b] = path_data[c*BS + b, 4q + r]
    pdx = nc.dram_tensor("pdx", [128, BS], dt.bfloat16, kind="ExternalInput").ap()
    bias = nc.dram_tensor("bias", [128, P, 2], dt.float32, kind="ExternalInput").ap()
    out = nc.dram_tensor(
        "out", [P, 128, 2, NT, TN], dt.bfloat16, kind="ExternalOutput"
    ).ap()

    with tile.TileContext(nc) as tc, ExitStack() as ctx:
        const = ctx.enter_context(tc.tile_pool(name="const", bufs=1))
        state = ctx.enter_context(tc.tile_pool(name="state", bufs=10))
        psum = ctx.enter_context(tc.tile_pool(name="psum", bufs=2, space="PSUM"))

        wx_sb = const.tile([128, P, 2, 128], dt.bfloat16)
        pdx_sb = const.tile([128, BS], dt.bfloat16)
        b_sb = const.tile([128, P, 2], dt.float32)
        wh_sb = const.tile([128, P, 2, 256], dt.bfloat16)
        warm_sb = const.tile([128, 64], dt.bfloat16)

        # Input DMAs on the sync HWDGE ring, ordered by first use; all
        # transfers are full 128-partition width.
        nc.sync.dma_start(out=wx_sb[:, 0:2, :, :], in_=wx[:, 0:2, :, :])
        nc.sync.dma_start(out=pdx_sb[:], in_=pdx[:])
        nc.sync.dma_start(out=wh_sb[:, 0:2, :, :], in_=wh[:, 0:2, :, :])
        nc.sync.dma_start(out=b_sb[:], in_=bias[:])
        nc.sync.dma_start(out=wx_sb[:, 2:8, :, :], in_=wx[:, 2:8, :, :])
        nc.sync.dma_start(out=wh_sb[:, 2:6, :, :], in_=wh[:, 2:6, :, :])
        nc.sync.dma_start(out=wx_sb[:, 8:16, :, :], in_=wx[:, 8:16, :, :])
        nc.sync.dma_start(out=wh_sb[:, 6:10, :, :], in_=wh[:, 6:10, :, :])
        nc.sync.dma_start(out=wx_sb[:, 16:24, :, :], in_=wx[:, 16:24, :, :])
        nc.sync.dma_start(out=wh_sb[:, 10:14, :, :], in_=wh[:, 10:14, :, :])
        nc.sync.dma_start(out=wx_sb[:, 24:32, :, :], in_=wx[:, 24:32, :, :])
        for g in range(5):
            sl = slice(14 + g * 4, min(14 + (g + 1) * 4, P))
            nc.sync.dma_start(out=wh_sb[:, sl, :, :], in_=wh[:, sl, :, :])

        # PE warmup: flip the HAM clock gate to 8/8 while inputs stream.
        nc.vector.memset(warm_sb[:], 0.0)
        warm_ps = psum.tile([128, NT, TN], dt.float32, tag="ps_m0", name="warm")
        for i in range(NWARM):
            nc.tensor.matmul(
                warm_ps[0:64, 0, 0:64],
                lhsT=warm_sb[:],
                rhs=warm_sb[:],
                start=True,
                stop=True,
                skip_group_check=True,
            )

        h_prev = None
        for p in range(P):
            ps = [
                psum.tile(
                    [128, NT, TN],
                    dt.float32,
                    tag=f"ps_m{m}",
                    name=f"ps_p{p}m{m}",
                )
                for m in range(2)
            ]
            # wp pass: K=128 matmuls, zero lhsT rows select step p. Opens
            # each accumulation group. Full-mode: the scheduler may hoist
            # these into the previous step's stream at no cost.
            for m in range(2):
                for t in range(NT):
                    nc.tensor.matmul(
                        ps[m][:, t, :],
                        lhsT=wx_sb[:, p, m, :],
                        rhs=pdx_sb[:, ts(t, TN)],
                        start=True,
                        stop=(p == 0),
                    )
            hn = state.tile(
                [128, 2, NT, TN], dt.bfloat16, tag="h", name=f"h_p{p}"
            )

            def relu_act(t):
                nc.scalar.activation(
                    hn[:, 0, t, :],
                    ps[0][:, t, :],
                    mybir.ActivationFunctionType.Relu,
                    bias=b_sb[:, p, 0:1],
                    scale=1.0,
                )

            def relu_dve(t):
                nc.vector.tensor_scalar(
                    hn[:, 1, t, :],
                    ps[1][:, t, :],
                    scalar1=b_sb[:, p, 1:2],
                    scalar2=0.0,
                    op0=mybir.AluOpType.add,
                    op1=mybir.AluOpType.max,
                )

            if p > 0:
                # t-outer; per t: (k0,m0),(k1,m0),(k0,m1),(k1,m1) so banks
                # close at slots 2/4/6/8; relu fires as each bank closes.
                # k0 matmuls consume ACT products, k1 consume DVE products.
                for t in range(NT):
                    for m in range(2):
                        for k in range(2):
                            nc.tensor.matmul(
                                ps[m][:, t, :],
                                lhsT=wh_sb[:, p, k, ts(m, 128)],
                                rhs=h_prev[:, k, t, :],
                                start=False,
                                stop=(k == 1),
                            )
                        if m == 0:
                            relu_act(t)
                        else:
                            relu_dve(t)
            else:
                for t in range(NT):
                    relu_act(t)
                    relu_dve(t)
            # outputs: t=0 halves on the idle Sync ring, t=1 on SWDGE;
            # the last two steps split 3 ways (+scalar) to cut the drain
            if p >= P - 2:
                nc.sync.dma_start(out=out[p, :, :, 0, :], in_=hn[:, :, 0, :])
                nc.gpsimd.dma_start(out=out[p, :, 0, 1, :], in_=hn[:, 0, 1, :])
                nc.scalar.dma_start(out=out[p, :, 1, 1, :], in_=hn[:, 1, 1, :])
            else:
                nc.sync.dma_start(out=out[p, :, :, 0, :], in_=hn[:, :, 0, :])
                nc.gpsimd.dma_start(out=out[p, :, :, 1, :], in_=hn[:, :, 1, :])
            h_prev = hn

    nc.compile()
    return nc


def _get_nc():
    if "nc" not in _CACHE:
        _CACHE["nc"] = _build_nc()
    return _CACHE["nc"]


def _pack_inputs(path_data, W, b):
    """Host-side packing into the DRAM layouts the kernel expects."""
    # lhsT for the two K=128 chunks: wh[kk, p, k, jj] = W[p, jj, 128k+kk]
    wh_np = np.ascontiguousarray(
        W[:, :, :H].reshape(P, H, 2, 128).transpose(3, 0, 2, 1)
    ).astype(BF16)
    # K=128 wp lhsT blocks, zero rows select the step:
    # wx[4q+r, p, m, j] = W[p, 128m+j, 256+r] if q == p else 0
    wx_np = np.zeros((128, P, 2, 128), dtype=BF16)
    wxs = W[:, :, H:].reshape(P, 2, 128, PD).transpose(3, 0, 1, 2).astype(BF16)
    for p in range(P):
        wx_np[4 * p : 4 * p + 4, p] = wxs[:, p]
    # bias[j, p, m] = b[p, 128m+j]
    b_np = np.ascontiguousarray(b.reshape(P, 2, 128).transpose(2, 0, 1)).astype(
        np.float32
    )
    # per-core rhs for the wp pass: pdx[4q+r, bb] = path_data[c*BS+bb, 4q+r]
    pdx_all = [
        np.ascontiguousarray(path_data[c * BS : (c + 1) * BS].T).astype(BF16)
        for c in range(NCORES)
    ]
    return wh_np, wx_np, b_np, pdx_all


def _make_in_maps(path_data, W, b):
    wh_np, wx_np, b_np, pdx_all = _pack_inputs(path_data, W, b)
    return [
        {"wh": wh_np, "wx": wx_np, "bias": b_np, "pdx": pdx_all[c]}
        for c in range(NCORES)
    ]


def _unpack_out(results):
    # out[p, jj, m, t, bb] -> full[c*BS + t*TN + bb, p*256 + m*128 + jj]
    return np.concatenate(
        [
            np.asarray(r["out"])
            .transpose(3, 4, 0, 2, 1)
            .reshape(BS, P * H)
            .astype(np.float32)
            for r in results
        ],
        axis=0,
    )


def kernel(path_data, W, b):
    from concourse.bass_utils import run_bass_kernel_spmd

    path_data = np.asarray(path_data, dtype=np.float32)
    W = np.asarray(W, dtype=np.float32)
    b = np.asarray(b, dtype=np.float32)

    in_maps = _make_in_maps(path_data, W, b)
    nc = _get_nc()
    res = run_bass_kernel_spmd(nc, in_maps, core_ids=list(range(NCORES)))
    return _unpack_out(res.results)
